# revision 34
# baseline (speedup 1.0000x reference)
"""Trainium2 Bass kernel for one GPT-style transformer block.

Problem: B=8, T=1024, C=768, NH=12 heads (HD=64), pre-LN attention + MLP,
key-padding mask, tanh-gelu.  Sharding: data-parallel over batch — each of
the 8 NeuronCores processes one batch element end-to-end (no collectives).

Trivial path (unit LN gains / zero biases — what setup_inputs() generates)
uses fp8e4m3 DoubleRow matmuls (2 contraction chunks per PE pass) for the
QKV, attention-proj and fc2 matmuls, plus a linearized-gelu decomposition
that keeps the error in budget:

  - h1 = LN1(x) stored fp8 as 8*h (scale folded into rstd via eps/64 trick);
    W_attn quantized host-side to fp8(16*W).  q,k stored bf16 at 128x scale
    (exp scale becomes 2^-17); v stored in vext at 128x with a 16*mask ones
    column so o = pav * (1/pav[64]) comes out as 8*vbar, which is exactly the
    fp8 scale wanted for oT.  proj: fp8(16*W_proj) DoubleRow; the 1/128
    descale is fused into the residual add via scalar_tensor_tensor.
  - MLP: gelu(z) = c*z + d + r(z), with per-column least-squares (c, d)
    computed host-side from ||W_fc[:,j]|| via Gauss-Hermite.  The linear
    part goes through W12 = 32*(W_fc*diag(c))@W_fc2 in bf16 (768x768, cheap,
    accurate, bypasses fc1 error); d@W_fc2 is a host constant added to x;
    only the small residual r = gelu(z) - (c*z+d) is quantized fp8 and hits
    W_fc2 (fp8, 32x) with DoubleRow.  fc1 itself stays bf16 (its fp8 error
    would blow the tolerance).  Both parts accumulate into one PSUM tile;
    the 1/32 descale fuses into the final residual add.
  - Key compaction (unchanged): tokens permuted so unmasked keys come
    first; <=640 unmasked keys -> 5 of 8 key chunks processed.

General path (nonzero biases/gains) keeps the plain bf16 implementation.
"""

import numpy as np
import ml_dtypes

import concourse.bass as bass
import concourse.mybir as mybir
import concourse.tile as tile
from concourse import bacc
from concourse.bass import ds, ts
from concourse.masks import make_identity

F32 = mybir.dt.float32
BF16 = mybir.dt.bfloat16
F8 = mybir.dt.float8e4
AF = mybir.ActivationFunctionType
ALU = mybir.AluOpType
DR = mybir.MatmulPerfMode.DoubleRow

T, C, NH, HD = 1024, 768, 12, 64
TT = T // 128          # 8 token tiles
CC = C // 128          # 6 feature chunks
FC = (4 * C) // 128    # 24 ffn-hidden chunks
N_CORES = 8
EPS = 1e-5
EXP_SCALE = 0.125 / 16384.0   # 2^-17: q,k carry 128x scale each


def _bcast(ap_1d: bass.AP, p: int = 128) -> bass.AP:
    """Broadcast a 1-D DRAM AP across p partitions (zero partition stride)."""
    return bass.AP(tensor=ap_1d.tensor, offset=ap_1d.offset, ap=[[0, p]] + ap_1d.ap)


def build_bass(repeat: int = 1, trivial: bool = True, kt_chunks: int = 8) -> bass.Bass:
    if trivial:
        return _build_trivial_fp8(repeat, kt_chunks)
    return _build_general(repeat, kt_chunks)


# ====================================================================
# trivial path: fp8 DoubleRow + linearized MLP
# ====================================================================
def _build_trivial_fp8(repeat: int, kt_chunks: int) -> bass.Bass:
    KT = kt_chunks
    nc = bacc.Bacc(None)

    # weights arrive host-pre-arranged in SBUF layout [128, chunk, cols]:
    # every DMA below is a plain 2D contiguous copy on the hardware DGE
    # (no gpsimd descriptor generation, no rearrange)
    x_d = nc.dram_tensor("x", [T, C], F32, kind="ExternalInput")
    mask_d = nc.dram_tensor("mask01", [128, TT], F32, kind="ExternalInput")
    wattn_d = nc.dram_tensor("w_attn", [128, CC, 3 * C], F8, kind="ExternalInput")
    wproj_d = nc.dram_tensor("w_proj", [128, CC, C], F8, kind="ExternalInput")
    wfc_d = nc.dram_tensor("w_fc", [128, CC, 4 * C], F8, kind="ExternalInput")
    wfc2_d = nc.dram_tensor("w_fc2", [128, FC, C], F8, kind="ExternalInput")
    w12_d = nc.dram_tensor("w12", [128, CC, C], BF16, kind="ExternalInput")
    cvec_d = nc.dram_tensor("cvec", [128, FC], F32, kind="ExternalInput")
    dvec_d = nc.dram_tensor("dvec", [128, FC], F32, kind="ExternalInput")
    dout_d = nc.dram_tensor("dout", [C], F32, kind="ExternalInput")
    out_d = nc.dram_tensor("out", [T, C], F32, kind="ExternalOutput")

    with tile.TileContext(nc) as tc:
        from contextlib import ExitStack

        with ExitStack() as ctx:
            consts = ctx.enter_context(tc.tile_pool(name="consts", bufs=1))
            xpool = ctx.enter_context(tc.tile_pool(name="xpool", bufs=1))
            htmp_pool = ctx.enter_context(tc.tile_pool(name="htmp", bufs=3))
            stat_pool = ctx.enter_context(tc.tile_pool(name="stats", bufs=6))
            hT2_pool = ctx.enter_context(tc.tile_pool(name="hT2", bufs=1))
            wproj_pool = ctx.enter_context(tc.tile_pool(name="wproj", bufs=1))
            wfc_pool = ctx.enter_context(tc.tile_pool(name="wfc", bufs=1))
            ps_mm = ctx.enter_context(tc.tile_pool(name="ps_mm", bufs=2, space="PSUM"))
            ps_s = ctx.enter_context(tc.tile_pool(name="ps_s", bufs=3, space="PSUM"))
            ps_tr = ctx.enter_context(tc.tile_pool(name="ps_tr", bufs=2, space="PSUM"))

            # ---------------- constants ----------------
            ident = consts.tile([128, 128], BF16, name="ident")
            make_identity(nc, ident)
            mask_col = consts.tile([128, TT], F32, name="mask_col")
            nc.gpsimd.dma_start(out=mask_col, in_=mask_d[:, :])
            # LN1 feeds fp8 h at 8x: rstd' = 8/sigma via sqrt((var+eps)/64)
            eps64_t = consts.tile([128, 1], F32, name="eps64_t")
            nc.vector.memset(eps64_t, EPS / 64.0)
            eps_t = consts.tile([128, 1], F32, name="eps_t")
            nc.vector.memset(eps_t, EPS)
            c_col = consts.tile([128, FC], F32, name="c_col")
            nc.gpsimd.dma_start(out=c_col, in_=cvec_d[:, :])
            d_col = consts.tile([128, FC], F32, name="d_col")
            nc.gpsimd.dma_start(out=d_col, in_=dvec_d[:, :])
            dout_bc = consts.tile([128, C], F32, name="dout_bc")
            nc.gpsimd.dma_start(out=dout_bc, in_=_bcast(dout_d[:]))

            def layer_norm_to_hT(x_slice, hT, t, s8, hT_f8=None):
                """LN over C of one token tile -> bf16 -> PE transpose ->
                hT[:, c, t*128:...] (hT dtype may be fp8; copy converts).
                s8: emit 8*normalized (for fp8 targets).  hT_f8: optional
                fp8 copy derived from hT by the (otherwise idle) gpsimd
                engine, keeping the psum drain on ACT only."""
                stats = stat_pool.tile([128, 2, 6], F32, name="stats", tag="lnstats")
                for i in range(2):
                    nc.vector.bn_stats(out=stats[:, i, :], in_=x_slice[:, ts(i, 384)])
                mv = stat_pool.tile([128, 2], F32, name="mv", tag="lnmv")
                nc.vector.bn_aggr(out=mv, in_=stats)
                rstd = stat_pool.tile([128, 1], F32, name="rstd", tag="rstd")
                if s8:
                    nc.scalar.activation(
                        out=rstd, in_=mv[:, 1:2], func=AF.Sqrt,
                        bias=eps64_t[:, 0:1], scale=1.0 / 64.0,
                    )
                else:
                    nc.scalar.activation(
                        out=rstd, in_=mv[:, 1:2], func=AF.Sqrt, bias=eps_t[:, 0:1]
                    )
                nc.vector.reciprocal(rstd, rstd)
                hbf = htmp_pool.tile([128, C], BF16, name="hbf", tag="hbf")
                nc.vector.tensor_scalar(
                    out=hbf, in0=x_slice, scalar1=mv[:, 0:1], scalar2=rstd,
                    op0=ALU.subtract, op1=ALU.mult,
                )
                for c in range(CC):
                    ptr = ps_tr.tile([128, 128], BF16, name="ptr", tag="tr")
                    nc.tensor.transpose(ptr, hbf[:, ts(c, 128)], ident)
                    nc.scalar.copy(hT[:, c, ts(t, 128)], ptr)
                    if hT_f8 is not None:
                        nc.gpsimd.tensor_copy(hT_f8[:, c, ts(t, 128)], hT[:, c, ts(t, 128)])

            # ================= one full block (repeatable) =================
            for _rep in range(repeat):
              x_t = xpool.tile([128, TT, C], F32, name="x_t", tag="x_t")

              # ---- attention scope ----
              with ExitStack() as actx:
                wattn_pool = actx.enter_context(tc.tile_pool(name="wattn", bufs=1))
                hT_pool = actx.enter_context(tc.tile_pool(name="hT1", bufs=1))
                qkT_pool = actx.enter_context(tc.tile_pool(name="qkT", bufs=1))
                vext_pool = actx.enter_context(tc.tile_pool(name="vext", bufs=1))
                sT_pool = actx.enter_context(tc.tile_pool(name="sT", bufs=4))
                opool = actx.enter_context(tc.tile_pool(name="opool", bufs=1))

                # DMA issue order is engine-processing order: x0, x1 first
                # (LN critical path), then wattn pair 0 (first matmul
                # operand), then the rest interleaved.
                wattn_sb = wattn_pool.tile([128, CC, 3 * C], F8, name="wattn_sb")

                def wattn_dma(cp):
                    nc.sync.dma_start(
                        out=wattn_sb[:, 2 * cp : 2 * cp + 2, :],
                        in_=wattn_d[:, 2 * cp : 2 * cp + 2, :],
                    )

                for t in range(2):
                    # column-halves so bn_stats(i=0) starts half a DMA early
                    for i in range(2):
                        nc.sync.dma_start(
                            out=x_t[:, t, ts(i, 384)],
                            in_=x_d[ts(t, 128), ts(i, 384)],
                        )
                wattn_dma(0)
                for t in range(2, 4):
                    nc.sync.dma_start(out=x_t[:, t, :], in_=x_d[ts(t, 128), :])
                wattn_dma(1)
                for t in range(4, 6):
                    nc.sync.dma_start(out=x_t[:, t, :], in_=x_d[ts(t, 128), :])
                wattn_dma(2)
                for t in range(6, TT):
                    nc.sync.dma_start(out=x_t[:, t, :], in_=x_d[ts(t, 128), :])
                wproj_sb = wproj_pool.tile([128, CC, C], F8, name="wproj_sb")
                nc.sync.dma_start(out=wproj_sb, in_=wproj_d[:, :, :])
                # wfc lives in an outer pool (no address overlap with the
                # attention pools) so its DMA streams during attention and
                # fc1 never waits on it at the MLP seam.
                wfc_sb = wfc_pool.tile([128, CC, 4 * C], F8, name="wfc_sb")
                nc.sync.dma_start(out=wfc_sb, in_=wfc_d[:, :, :])

                h1T = hT_pool.tile([128, CC, T], F8, name="h1T", tag="hT")
                qT = qkT_pool.tile([128, CC, T], BF16, name="qT", tag="qT")
                kT = qkT_pool.tile([128, CC, KT * 128], BF16, name="kT", tag="kT")
                vext = vext_pool.tile([128, NH, KT, HD + 1], BF16, name="vext")

                # q^T / k^T feature-major via fp8 DoubleRow (3 c-pair passes);
                # 256-col phases so matmuls start after just 2 LN'd tiles
                def qk_phase(nqi, interleave=False):
                    for m in range(12):
                        dest = qT if m < 6 else kT
                        nlim = T if m < 6 else KT * 128
                        n0 = nqi * 256
                        nsz = min(256, nlim - n0)
                        if nsz <= 0:
                            if interleave:
                                drain(1)
                            continue
                        pq = ps_mm.tile([128, nsz], F32, name="pq", tag="mm")
                        for cp in range(CC // 2):
                            nc.tensor.matmul(
                                pq,
                                lhsT=wattn_sb[:, 2 * cp : 2 * cp + 2, ts(m, 128)],
                                rhs=h1T[:, 2 * cp : 2 * cp + 2, ds(n0, nsz)],
                                start=(cp == 0),
                                stop=(cp == CC // 2 - 1),
                                perf_mode=DR,
                            )
                        nc.vector.tensor_copy(dest[:, m % 6, ds(n0, nsz)], pq)
                        if interleave:
                            drain(1)

                # LN t0-t3 issued first so the DVE FIFO isn't blocked behind
                # qk psum-copies; qk_phase(p) only needs tiles 2p,2p+1
                for t in range(4):
                    layer_norm_to_hT(x_t[:, t, :], h1T, t, s8=True)
                # ---- software-pipelined scores / AV / proj ----
                # The exp pipeline (ACT) paces scores: each (kt, nq)
                # sub-chunk is 4 concurrent K=64 matmuls (disjoint PE
                # row-groups via auto tile_position) + 2 exps.  Sub-chunks
                # are queued as "fillers" and drained one at a time between
                # AV chains / v-part groups / qk blocks so the in-order PE
                # stream always has independent work while ACT catches up.
                sT_tiles = {}
                fillers = []

                def drain(n=1):
                    for _ in range(min(n, len(fillers))):
                        fillers.pop(0)()

                def pair_subchunk(hc, kt, nq):
                    pss = []
                    for hr in range(2):
                        ps = ps_s.tile([128, 512], F32, name="pss", tag="ss")
                        pss.append(ps)
                        nc.tensor.matmul(
                            ps,
                            lhsT=kT[ds(hr * 64, 64), hc, ts(kt, 128)],
                            rhs=qT[ds(hr * 64, 64), hc, ts(nq, 512)],
                            start=True,
                            stop=True,
                        )
                    for hr in range(2):
                        nc.scalar.activation(
                            out=sT_tiles[2 * hc + hr][:, kt, ts(nq, 512)],
                            in_=pss[hr],
                            func=AF.Exp,
                            scale=EXP_SCALE,
                        )

                def queue_pair(hc, nqs=(0, 1)):
                    for hr in range(2):
                        if 2 * hc + hr not in sT_tiles:
                            sT_tiles[2 * hc + hr] = sT_pool.tile(
                                [128, KT, T], BF16, name="sT", tag="sT"
                            )
                    for nq in nqs:
                        for kt in range(KT):
                            fillers.append(
                                lambda hc=hc, kt=kt, nq=nq: pair_subchunk(hc, kt, nq)
                            )

                qk_phase(0)
                for t in range(4, 6):
                    layer_norm_to_hT(x_t[:, t, :], h1T, t, s8=True)
                qk_phase(1)
                for t in range(6, TT):
                    layer_norm_to_hT(x_t[:, t, :], h1T, t, s8=True)
                qk_phase(2)
                # nq=0 sub-chunks only need qT[:, :, 0:512] (phases 0-1) and
                # kT fully (phase 2) -> interleave them into phase 3
                queue_pair(0, nqs=(0,))
                queue_pair(1, nqs=(0,))
                qk_phase(3, interleave=True)
                queue_pair(0, nqs=(1,))
                queue_pair(1, nqs=(1,))

                # ones column = 16*mask: AV denominator = 16*sum(p), so
                # o_t = pav * (1/pav[64]) = 8 * vbar  (v stored at 128x)
                nc.vector.memset(vext[:, :, :, HD : HD + 1], 16.0)
                for t in range(KT):
                    nc.vector.tensor_scalar_mul(
                        out=vext[:, :, t, HD : HD + 1],
                        in0=vext[:, :, t, HD : HD + 1],
                        scalar1=mask_col[:, t : t + 1],
                    )

                # v token-major at 128x scale, masked, scattered per head
                # (256-wide chunks: DoubleRow moving limit is 2*256 elems)
                for t in range(KT):
                    for n0 in range(0, C, 256):
                        pv = ps_mm.tile([128, 256], F32, name="pv", tag="mm")
                        for cp in range(CC // 2):
                            nc.tensor.matmul(
                                pv,
                                lhsT=h1T[:, 2 * cp : 2 * cp + 2, ts(t, 128)],
                                rhs=wattn_sb[:, 2 * cp : 2 * cp + 2, ds(2 * C + n0, 256)],
                                start=(cp == 0),
                                stop=(cp == CC // 2 - 1),
                                perf_mode=DR,
                            )
                        h0, h1 = n0 // HD, (n0 + 256) // HD
                        pv_h = pv.rearrange("p (h d) -> p h d", d=HD)
                        nc.vector.tensor_scalar_mul(
                            out=vext[:, h0:h1, t, 0:HD], in0=pv_h,
                            scalar1=mask_col[:, t : t + 1],
                        )
                        drain(1)

                o_t = opool.tile([128, TT, C], BF16, name="o_t", tag="op")
                oT = qkT_pool.tile([128, CC, T], F8, name="oT", tag="oT")

                def proj_tile(cp, t):
                    # partial proj contribution of oT c-pair cp for tile t,
                    # folded into x (the 1/128 descale distributes over the
                    # sum).  Queued as a filler once chunk cp of oT exists.
                    for n0 in range(0, C, 256):
                        pp = ps_mm.tile([128, 256], F32, name="pp", tag="mm")
                        nc.tensor.matmul(
                            pp,
                            lhsT=oT[:, 2 * cp : 2 * cp + 2, ts(t, 128)],
                            rhs=wproj_sb[:, 2 * cp : 2 * cp + 2, ds(n0, 256)],
                            start=True,
                            stop=True,
                            perf_mode=DR,
                        )
                        nc.vector.scalar_tensor_tensor(
                            out=x_t[:, t, ds(n0, 256)], in0=pp,
                            scalar=1.0 / 128.0, in1=x_t[:, t, ds(n0, 256)],
                            op0=ALU.mult, op1=ALU.add,
                        )

                def queue_proj(cp):
                    for t in range(TT):
                        fillers.append(lambda cp=cp, t=t: proj_tile(cp, t))

                # AV: 4 token tiles accumulate into one psum bank, then one
                # batched reciprocal + one broadcast multiply normalize all
                # four (instead of per-tile recip+mul DVE pairs).
                for h in range(NH):
                    sT = sT_tiles.pop(h)
                    for half in range(2):
                        pav8 = ps_mm.tile(
                            [128, 4, 128], F32, name="pav8", tag="av", bufs=1
                        )
                        for i in range(4):
                            tq = half * 4 + i
                            for kt in range(KT):
                                nc.tensor.matmul(
                                    pav8[:, i, 0 : HD + 1],
                                    lhsT=sT[:, kt, ts(tq, 128)],
                                    rhs=vext[:, h, kt, :],
                                    start=(kt == 0),
                                    stop=(kt == KT - 1),
                                )
                            drain(1)
                        rec4 = stat_pool.tile([128, 4], F32, name="rec4", tag="rec")
                        nc.vector.reciprocal(rec4, pav8[:, :, HD : HD + 1])
                        nc.vector.tensor_tensor(
                            out=o_t[:, ds(half * 4, 4), ts(h, HD)],
                            in0=pav8[:, :, 0:HD],
                            in1=rec4[:, :].unsqueeze(2).broadcast_to([128, 4, HD]),
                            op=ALU.mult,
                        )
                        if h % 2 == 1:
                            # both heads of chunk h//2 done for these tiles:
                            # transpose now so proj can chase the AV sweep
                            c = h // 2
                            for i in range(4):
                                tq = half * 4 + i
                                ptr = ps_tr.tile([128, 128], BF16, name="ptr2", tag="tr")
                                nc.tensor.transpose(ptr, o_t[:, tq, ts(c, 128)], ident)
                                nc.vector.tensor_copy(oT[:, c, ts(tq, 128)], ptr)
                        drain(1)
                    if h % 2 == 1:
                        if h + 3 < NH:
                            queue_pair((h + 3) // 2)
                        if h % 4 == 3:
                            # oT c-pair h//4 complete (heads h-3..h) -> its
                            # proj contribution can chase the AV sweep
                            queue_proj(h // 4)

                # tail: the last proj chunk drains 1:1 with LN2 tiles.  LN2
                # emits 8*normalized twice: bf16 via ACT (feeds the exact
                # W12 linear path) and an fp8 gpsimd copy (feeds fc1 DR).
                assert len(fillers) == TT, f"filler backlog {len(fillers)}"
                h2f = hT2_pool.tile([128, CC, T], F8, name="h2f", tag="hT2f")
                h2b = hT2_pool.tile([128, CC, T], BF16, name="h2b", tag="hT2b")
                for t in range(TT):
                    drain(2)
                    layer_norm_to_hT(x_t[:, t, :], h2b, t, s8=True, hT_f8=h2f)
                assert not fillers

              # ---- MLP scope ----
              with ExitStack() as mctx:
                wfc2_pool = mctx.enter_context(tc.tile_pool(name="wfc2", bufs=1))
                w12_pool = mctx.enter_context(tc.tile_pool(name="w12", bufs=1))
                rT_pool = mctx.enter_context(tc.tile_pool(name="rT", bufs=2))
                gz_pool = mctx.enter_context(tc.tile_pool(name="gz", bufs=6))
                outsb_pool = mctx.enter_context(tc.tile_pool(name="outsb", bufs=2))

                wfc2_sb = wfc2_pool.tile([128, FC, C], F8, name="wfc2_sb")
                nc.sync.dma_start(out=wfc2_sb, in_=wfc2_d[:, :, :])
                w12_sb = w12_pool.tile([128, CC, C], BF16, name="w12_sb")
                nc.sync.dma_start(out=w12_sb, in_=w12_d[:, :, :])

                # x += dout (the d@W_fc2 constant) once per tile; x_t was
                # already consumed by LN2 above so this is safe
                for t in range(TT):
                    nc.vector.tensor_add(x_t[:, t, :], x_t[:, t, :], dout_bc)

                def fc1_m(half, rT, m):
                    # fc1 in fp8 DoubleRow: psum = 128*z (8x h2 * 16x W_fc);
                    # the 1/128 descale folds into the gelu scale and the
                    # host-prescaled c vector.  N=256 chunks (DR moving
                    # limit), sequential accumulation groups per chunk.
                    pf = ps_mm.tile([128, 512], F32, name="pf", tag="mm")
                    for ni in range(2):
                        for cp in range(CC // 2):
                            nc.tensor.matmul(
                                pf[:, ts(ni, 256)],
                                lhsT=wfc_sb[:, 2 * cp : 2 * cp + 2, ts(m, 128)],
                                rhs=h2f[:, 2 * cp : 2 * cp + 2,
                                        ds(half * 512 + ni * 256, 256)],
                                start=(cp == 0),
                                stop=(cp == CC // 2 - 1),
                                perf_mode=DR,
                            )
                    g = gz_pool.tile([128, 512], BF16, name="g", tag="g")
                    nc.scalar.activation(
                        out=g, in_=pf, func=AF.Gelu_apprx_tanh, scale=1.0 / 128.0
                    )
                    zc = gz_pool.tile([128, 512], BF16, name="zc", tag="zc")
                    nc.vector.tensor_scalar(
                        out=zc, in0=pf, scalar1=c_col[:, m : m + 1],
                        scalar2=d_col[:, m : m + 1], op0=ALU.mult, op1=ALU.add,
                    )
                    # SBUF-only subtract, alternated between the idle
                    # gpsimd engine and DVE so neither paces the fc1 loop
                    eng = nc.gpsimd if m % 2 == 0 else nc.vector
                    eng.tensor_sub(rT[:, m, :], g, zc)

                def fc2_tile(half, rT, i):
                    t = half * 4 + i
                    outsb = outsb_pool.tile([128, C], F32, name="outsb", tag="outsb")
                    for n0 in range(0, C, 256):
                        pf2 = ps_mm.tile([128, 256], F32, name="pf2", tag="mm")
                        # linear part: 8*h2 @ 4*W12 (bf16, 32x total)
                        for c in range(CC):
                            nc.tensor.matmul(
                                pf2,
                                lhsT=h2b[:, c, ts(t, 128)],
                                rhs=w12_sb[:, c, ds(n0, 256)],
                                start=(c == 0),
                                stop=False,
                            )
                        # residual part: r @ W_fc2 (fp8 DR, 32x)
                        for mp in range(FC // 2):
                            nc.tensor.matmul(
                                pf2,
                                lhsT=rT[:, 2 * mp : 2 * mp + 2, ts(i, 128)],
                                rhs=wfc2_sb[:, 2 * mp : 2 * mp + 2, ds(n0, 256)],
                                start=False,
                                stop=(mp == FC // 2 - 1),
                                perf_mode=DR,
                            )
                        nc.vector.scalar_tensor_tensor(
                            out=outsb[:, ds(n0, 256)], in0=pf2,
                            scalar=1.0 / 32.0, in1=x_t[:, t, ds(n0, 256)],
                            op0=ALU.mult, op1=ALU.add,
                        )
                        nc.sync.dma_start(
                            out=out_d[ts(t, 128), ds(n0, 256)],
                            in_=outsb[:, ds(n0, 256)],
                        )

                # half-0 fc1, then half-1 fc1 interleaved with half-0 fc2
                # tiles so PE never waits on the Pool r-subtract pipeline
                rT0 = rT_pool.tile([128, FC, 512], F8, name="rT0", tag="rT")
                for m in range(FC):
                    fc1_m(0, rT0, m)
                rT1 = rT_pool.tile([128, FC, 512], F8, name="rT1", tag="rT")
                for i in range(4):
                    for m in range(6 * i, 6 * i + 6):
                        fc1_m(1, rT1, m)
                    fc2_tile(0, rT0, i)
                for i in range(4):
                    fc2_tile(1, rT1, i)

    return nc


# ====================================================================
# general path: plain bf16 (biases/gains applied) — unchanged baseline
# ====================================================================
def _build_general(repeat: int, kt_chunks: int) -> bass.Bass:
    KT = kt_chunks
    nc = bacc.Bacc(None)

    x_d = nc.dram_tensor("x", [T, C], F32, kind="ExternalInput")
    mask_d = nc.dram_tensor("mask01", [T], F32, kind="ExternalInput")
    wattn_d = nc.dram_tensor("w_attn", [C, 3 * C], BF16, kind="ExternalInput")
    wproj_d = nc.dram_tensor("w_proj", [C, C], BF16, kind="ExternalInput")
    wfc_d = nc.dram_tensor("w_fc", [C, 4 * C], BF16, kind="ExternalInput")
    wfc2_d = nc.dram_tensor("w_fc2", [4 * C, C], BF16, kind="ExternalInput")
    ln1g_d = nc.dram_tensor("ln1_g", [C], F32, kind="ExternalInput")
    ln1b_d = nc.dram_tensor("ln1_b", [C], F32, kind="ExternalInput")
    ln2g_d = nc.dram_tensor("ln2_g", [C], F32, kind="ExternalInput")
    ln2b_d = nc.dram_tensor("ln2_b", [C], F32, kind="ExternalInput")
    battn_d = nc.dram_tensor("b_attn", [3 * C], F32, kind="ExternalInput")
    bproj_d = nc.dram_tensor("b_proj", [C], F32, kind="ExternalInput")
    bfc_d = nc.dram_tensor("b_fc", [4 * C], F32, kind="ExternalInput")
    bfc2_d = nc.dram_tensor("b_fc2", [C], F32, kind="ExternalInput")
    out_d = nc.dram_tensor("out", [T, C], F32, kind="ExternalOutput")

    with tile.TileContext(nc) as tc:
        from contextlib import ExitStack

        with ExitStack() as ctx:
            consts = ctx.enter_context(tc.tile_pool(name="consts", bufs=1))
            xpool = ctx.enter_context(tc.tile_pool(name="xpool", bufs=1))
            htmp_pool = ctx.enter_context(tc.tile_pool(name="htmp", bufs=3))
            stat_pool = ctx.enter_context(tc.tile_pool(name="stats", bufs=6))
            hT2_pool = ctx.enter_context(tc.tile_pool(name="hT2", bufs=1))
            wproj_pool = ctx.enter_context(tc.tile_pool(name="wproj", bufs=1))
            ps_mm = ctx.enter_context(tc.tile_pool(name="ps_mm", bufs=2, space="PSUM"))
            ps_s = ctx.enter_context(tc.tile_pool(name="ps_s", bufs=2, space="PSUM"))
            ps_tr = ctx.enter_context(tc.tile_pool(name="ps_tr", bufs=2, space="PSUM"))

            ident = consts.tile([128, 128], BF16, name="ident")
            make_identity(nc, ident)
            mask_col = consts.tile([128, TT], F32, name="mask_col")
            nc.gpsimd.dma_start(out=mask_col, in_=mask_d[:].rearrange("(t p) -> p t", p=128))
            eps_t = consts.tile([128, 1], F32, name="eps_t")
            nc.vector.memset(eps_t, EPS)

            g1_bc = consts.tile([128, C], F32, name="g1_bc")
            b1_bc = consts.tile([128, C], F32, name="b1_bc")
            g2_bc = consts.tile([128, C], F32, name="g2_bc")
            b2_bc = consts.tile([128, C], F32, name="b2_bc")
            battnv_bc = consts.tile([128, C], F32, name="battnv_bc")
            bproj_bc = consts.tile([128, C], F32, name="bproj_bc")
            bfc2_bc = consts.tile([128, C], F32, name="bfc2_bc")
            nc.gpsimd.dma_start(out=g1_bc, in_=_bcast(ln1g_d[:]))
            nc.gpsimd.dma_start(out=b1_bc, in_=_bcast(ln1b_d[:]))
            nc.gpsimd.dma_start(out=g2_bc, in_=_bcast(ln2g_d[:]))
            nc.gpsimd.dma_start(out=b2_bc, in_=_bcast(ln2b_d[:]))
            nc.gpsimd.dma_start(out=battnv_bc, in_=_bcast(battn_d[ds(2 * C, C)]))
            nc.gpsimd.dma_start(out=bproj_bc, in_=_bcast(bproj_d[:]))
            nc.gpsimd.dma_start(out=bfc2_bc, in_=_bcast(bfc2_d[:]))
            battn_qk = consts.tile([128, 12], F32, name="battn_qk")
            nc.gpsimd.dma_start(
                out=battn_qk,
                in_=battn_d[ds(0, 2 * C)].rearrange("(m p) -> p m", p=128),
            )
            bfc_col = consts.tile([128, FC], F32, name="bfc_col")
            nc.gpsimd.dma_start(
                out=bfc_col, in_=bfc_d[:].rearrange("(m p) -> p m", p=128)
            )

            def layer_norm_to_hT(x_slice, g_bc, b_bc, hT, t):
                stats = stat_pool.tile([128, 2, 6], F32, name="stats", tag="lnstats")
                for i in range(2):
                    nc.vector.bn_stats(out=stats[:, i, :], in_=x_slice[:, ts(i, 384)])
                mv = stat_pool.tile([128, 2], F32, name="mv", tag="lnmv")
                nc.vector.bn_aggr(out=mv, in_=stats)
                rstd = stat_pool.tile([128, 1], F32, name="rstd", tag="rstd")
                nc.scalar.activation(out=rstd, in_=mv[:, 1:2], func=AF.Sqrt, bias=eps_t[:, 0:1])
                nc.vector.reciprocal(rstd, rstd)
                hbf = htmp_pool.tile([128, C], BF16, name="hbf", tag="hbf")
                htmp = htmp_pool.tile([128, C], F32, name="htmp", tag="htmp")
                nc.vector.tensor_scalar(
                    out=htmp, in0=x_slice, scalar1=mv[:, 0:1], scalar2=rstd,
                    op0=ALU.subtract, op1=ALU.mult,
                )
                nc.vector.tensor_mul(htmp, htmp, g_bc)
                nc.vector.tensor_add(hbf, htmp, b_bc)
                for c in range(CC):
                    ptr = ps_tr.tile([128, 128], BF16, name="ptr", tag="tr")
                    nc.tensor.transpose(ptr, hbf[:, ts(c, 128)], ident)
                    nc.scalar.copy(hT[:, c, ts(t, 128)], ptr)

            for _rep in range(repeat):
              x_t = xpool.tile([128, TT, C], F32, name="x_t", tag="x_t")
              for t in range(TT):
                  nc.sync.dma_start(out=x_t[:, t, :], in_=x_d[ts(t, 128), :])

              with ExitStack() as actx:
                wattn_pool = actx.enter_context(tc.tile_pool(name="wattn", bufs=1))
                hT_pool = actx.enter_context(tc.tile_pool(name="hT1", bufs=1))
                qkT_pool = actx.enter_context(tc.tile_pool(name="qkT", bufs=1))
                vext_pool = actx.enter_context(tc.tile_pool(name="vext", bufs=1))
                sT_pool = actx.enter_context(tc.tile_pool(name="sT", bufs=3))
                opool = actx.enter_context(tc.tile_pool(name="opool", bufs=1))

                wattn_sb = wattn_pool.tile([128, CC, 3 * C], BF16, name="wattn_sb")
                nc.gpsimd.dma_start(
                    out=wattn_sb, in_=wattn_d[:, :].rearrange("(c p) n -> p c n", p=128)
                )
                wproj_sb = wproj_pool.tile([128, CC, C], BF16, name="wproj_sb")
                nc.gpsimd.dma_start(
                    out=wproj_sb, in_=wproj_d[:, :].rearrange("(c p) n -> p c n", p=128)
                )

                h1T = hT_pool.tile([128, CC, T], BF16, name="h1T", tag="hT")
                qT = qkT_pool.tile([128, CC, T], BF16, name="qT", tag="qT")
                kT = qkT_pool.tile([128, CC, KT * 128], BF16, name="kT", tag="kT")
                vext = vext_pool.tile([128, NH, KT, HD + 1], BF16, name="vext")
                nc.vector.memset(vext[:, :, :, HD : HD + 1], 1.0)

                def qk_phase(nqi):
                    for m in range(12):
                        dest = qT if m < 6 else kT
                        nlim = T if m < 6 else KT * 128
                        n0 = nqi * 512
                        nsz = min(512, nlim - n0)
                        if nsz <= 0:
                            continue
                        pq = ps_mm.tile([128, nsz], F32, name="pq", tag="mm")
                        for c in range(CC):
                            nc.tensor.matmul(
                                pq,
                                lhsT=wattn_sb[:, c, ts(m, 128)],
                                rhs=h1T[:, c, ds(n0, nsz)],
                                start=(c == 0),
                                stop=(c == CC - 1),
                            )
                        nc.scalar.activation(
                            out=dest[:, m % 6, ds(n0, nsz)], in_=pq,
                            func=AF.Identity, bias=battn_qk[:, m : m + 1],
                        )

                for t in range(4):
                    layer_norm_to_hT(x_t[:, t, :], g1_bc, b1_bc, h1T, t)
                qk_phase(0)
                for t in range(4, TT):
                    layer_norm_to_hT(x_t[:, t, :], g1_bc, b1_bc, h1T, t)
                qk_phase(1)

                sT_tiles = {}

                def emit_scores(h):
                    hc, hr = divmod(h, 2)
                    r0 = hr * 64
                    sT = sT_pool.tile([128, KT, T], BF16, name="sT", tag="sT")
                    sT_tiles[h] = sT
                    for kt in range(KT):
                        pss = ps_s.tile([128, 2, 512], F32, name="pss", tag="ss")
                        for nq in range(2):
                            nc.tensor.matmul(
                                pss[:, nq, :],
                                lhsT=kT[ds(r0, 64), hc, ts(kt, 128)],
                                rhs=qT[ds(r0, 64), hc, ts(nq, 512)],
                                start=True,
                                stop=True,
                            )
                        nc.scalar.activation(
                            out=sT[:, kt, :],
                            in_=pss.rearrange("p a b -> p (a b)"),
                            func=AF.Exp,
                            scale=0.125,
                        )

                emit_scores(0)
                emit_scores(1)
                emit_scores(2)

                for t in range(KT):
                    for n0, nsz in ((0, 512), (512, 256)):
                        pv = ps_mm.tile([128, nsz], F32, name="pv", tag="mm")
                        for c in range(CC):
                            nc.tensor.matmul(
                                pv,
                                lhsT=h1T[:, c, ts(t, 128)],
                                rhs=wattn_sb[:, c, ds(2 * C + n0, nsz)],
                                start=(c == 0),
                                stop=(c == CC - 1),
                            )
                        h0, h1 = n0 // HD, (n0 + nsz) // HD
                        pv_h = pv.rearrange("p (h d) -> p h d", d=HD)
                        nc.vector.tensor_add(
                            out=vext[:, h0:h1, t, 0:HD], in0=pv_h,
                            in1=battnv_bc[:, ds(n0, nsz)].rearrange(
                                "p (h d) -> p h d", d=HD
                            ),
                        )
                for t in range(KT):
                    nc.vector.tensor_scalar_mul(
                        out=vext[:, :, t, :], in0=vext[:, :, t, :],
                        scalar1=mask_col[:, t : t + 1],
                    )

                o_t = opool.tile([128, TT, C], BF16, name="o_t", tag="op")
                oT = qkT_pool.tile([128, CC, T], BF16, name="oT", tag="oT")
                for h in range(NH):
                    sT = sT_tiles.pop(h)
                    for tq in range(TT):
                        pav = ps_mm.tile([128, HD + 1], F32, name="pav", tag="mm")
                        for kt in range(KT):
                            nc.tensor.matmul(
                                pav,
                                lhsT=sT[:, kt, ts(tq, 128)],
                                rhs=vext[:, h, kt, :],
                                start=(kt == 0),
                                stop=(kt == KT - 1),
                            )
                        rec = stat_pool.tile([128, 1], F32, name="rec", tag="rec")
                        nc.vector.reciprocal(rec, pav[:, HD : HD + 1])
                        nc.vector.tensor_scalar_mul(
                            out=o_t[:, tq, ts(h, HD)], in0=pav[:, 0:HD], scalar1=rec
                        )
                    if h + 3 < NH:
                        emit_scores(h + 3)
                    if h % 2 == 1:
                        c = h // 2
                        for t in range(TT):
                            ptr = ps_tr.tile([128, 128], BF16, name="ptr2", tag="tr")
                            nc.tensor.transpose(ptr, o_t[:, t, ts(c, 128)], ident)
                            nc.vector.tensor_copy(oT[:, c, ts(t, 128)], ptr)

                h2T = hT2_pool.tile([128, CC, T], BF16, name="h2T", tag="hT2")
                for grp in range(2):
                    for t in range(grp * 4, grp * 4 + 4):
                        for n0, nsz in ((0, 512), (512, 256)):
                            pp = ps_mm.tile([128, nsz], F32, name="pp", tag="mm")
                            for c in range(CC):
                                nc.tensor.matmul(
                                    pp,
                                    lhsT=oT[:, c, ts(t, 128)],
                                    rhs=wproj_sb[:, c, ds(n0, nsz)],
                                    start=(c == 0),
                                    stop=(c == CC - 1),
                                )
                            nc.vector.tensor_add(pp, pp, bproj_bc[:, ds(n0, nsz)])
                            nc.vector.tensor_add(
                                x_t[:, t, ds(n0, nsz)], x_t[:, t, ds(n0, nsz)], pp
                            )
                    for t in range(grp * 4, grp * 4 + 4):
                        layer_norm_to_hT(x_t[:, t, :], g2_bc, b2_bc, h2T, t)

              with ExitStack() as mctx:
                wfc_pool = mctx.enter_context(tc.tile_pool(name="wfc", bufs=1))
                wfc2_pool = mctx.enter_context(tc.tile_pool(name="wfc2", bufs=1))
                aT_pool = mctx.enter_context(tc.tile_pool(name="aT", bufs=1))
                outsb_pool = mctx.enter_context(tc.tile_pool(name="outsb", bufs=2))

                wfc_sb = wfc_pool.tile([128, CC, 4 * C], BF16, name="wfc_sb")
                nc.gpsimd.dma_start(
                    out=wfc_sb, in_=wfc_d[:, :].rearrange("(c p) n -> p c n", p=128)
                )
                wfc2_sb = wfc2_pool.tile([128, FC, C], BF16, name="wfc2_sb")
                nc.gpsimd.dma_start(
                    out=wfc2_sb, in_=wfc2_d[:, :].rearrange("(m p) n -> p m n", p=128)
                )

                for half in range(2):
                    aT = aT_pool.tile([128, FC, 512], BF16, name="aT", tag="aT")
                    for m in range(FC):
                        pf = ps_mm.tile([128, 512], F32, name="pf", tag="mm")
                        for c in range(CC):
                            nc.tensor.matmul(
                                pf,
                                lhsT=wfc_sb[:, c, ts(m, 128)],
                                rhs=h2T[:, c, ds(half * 512, 512)],
                                start=(c == 0),
                                stop=(c == CC - 1),
                            )
                        nc.scalar.activation(
                            out=aT[:, m, :], in_=pf, func=AF.Gelu_apprx_tanh,
                            bias=bfc_col[:, m : m + 1],
                        )
                    for i in range(4):
                        t = half * 4 + i
                        outsb = outsb_pool.tile([128, C], F32, name="outsb", tag="outsb")
                        for n0, nsz in ((0, 512), (512, 256)):
                            pf2 = ps_mm.tile([128, nsz], F32, name="pf2", tag="mm")
                            for m in range(FC):
                                nc.tensor.matmul(
                                    pf2,
                                    lhsT=aT[:, m, ts(i, 128)],
                                    rhs=wfc2_sb[:, m, ds(n0, nsz)],
                                    start=(m == 0),
                                    stop=(m == FC - 1),
                                )
                            nc.vector.tensor_add(pf2, pf2, bfc2_bc[:, ds(n0, nsz)])
                            nc.vector.tensor_add(
                                outsb[:, ds(n0, nsz)], x_t[:, t, ds(n0, nsz)], pf2
                            )
                        nc.sync.dma_start(out=out_d[ts(t, 128), :], in_=outsb)

    return nc


_NC_CACHE = {}

COMPACT_KT = 5  # attention processes 5*128 = 640 keys; guarded in kernel()


def _get_nc(trivial: bool = True, kt_chunks: int = COMPACT_KT) -> bass.Bass:
    key = (trivial, kt_chunks)
    if key not in _NC_CACHE:
        nc = build_bass(trivial=trivial, kt_chunks=kt_chunks)
        nc.finalize()
        _NC_CACHE[key] = nc
    return _NC_CACHE[key]


TRACE = False
LAST_RESULTS = None
LAST_IN_MAPS = None


def _f8(a: np.ndarray, scale: float) -> np.ndarray:
    """Host-side quantize to TRN-compatible e4m3 (clip +-240) after scaling."""
    s = np.clip(a.astype(np.float32) * scale, -240.0, 240.0)
    return np.ascontiguousarray(s.astype(ml_dtypes.float8_e4m3))


def _arr(w: np.ndarray) -> np.ndarray:
    """[K, N] -> SBUF layout [128, K//128, N] (chunked along K)."""
    k, n = w.shape
    return np.ascontiguousarray(w.reshape(k // 128, 128, n).transpose(1, 0, 2))


def _col(v: np.ndarray) -> np.ndarray:
    """[M] -> [128, M//128] (per-partition column layout)."""
    return np.ascontiguousarray(v.reshape(-1, 128).T)


def _gelu64(a):
    return 0.5 * a * (1.0 + np.tanh(0.7978845608028654 * (a + 0.044715 * a**3)))


def _mlp_linearization(W_fc: np.ndarray, W_fc2: np.ndarray):
    """Per-column least-squares (c, d) for gelu under z_j ~ N(0, sigma_j),
    plus the 4x-scaled bf16 linear weight W12 (8x h2 * 4x W12 = 32x psum)
    and output constant dout.  c is pre-divided by 128 (fc1 psum carries
    128*z from the fp8 DoubleRow matmul)."""
    W1 = W_fc.astype(np.float64)
    W2 = W_fc2.astype(np.float64)
    gh_x, gh_w = np.polynomial.hermite_e.hermegauss(63)
    gh_w = gh_w / gh_w.sum()
    sigma = np.linalg.norm(W1, axis=0)
    sigma = np.maximum(sigma, 1e-12)
    zg = sigma[:, None] * gh_x[None, :]
    gz = _gelu64(zg)
    d_vec = (gz * gh_w[None, :]).sum(1)
    c_vec = (zg * gz * gh_w[None, :]).sum(1) / (sigma**2)
    W12 = (W1 * c_vec[None, :]) @ W2
    dout = d_vec @ W2
    return (
        np.ascontiguousarray((c_vec / 128.0).astype(np.float32)),
        np.ascontiguousarray(d_vec.astype(np.float32)),
        np.ascontiguousarray((4.0 * W12).astype(np.float32).astype(ml_dtypes.bfloat16)),
        np.ascontiguousarray(dout.astype(np.float32)),
    )


def kernel(**inputs) -> np.ndarray:
    global LAST_RESULTS, LAST_IN_MAPS

    f32 = lambda a: np.ascontiguousarray(np.asarray(a, dtype=np.float32))
    bf = lambda a: np.ascontiguousarray(
        np.asarray(a, dtype=np.float32).astype(ml_dtypes.bfloat16)
    )

    x = f32(inputs["x"])                       # [8, 1024, 768]
    mask = np.asarray(inputs["attn_mask"])     # [8, 1024] int32

    lng1, lnb1 = f32(inputs["ln1_g"]), f32(inputs["ln1_b"])
    lng2, lnb2 = f32(inputs["ln2_g"]), f32(inputs["ln2_b"])
    ba, bp = f32(inputs["b_attn"]), f32(inputs["b_proj"])
    bf_, bf2 = f32(inputs["b_fc"]), f32(inputs["b_fc2"])
    trivial = bool(
        (lng1 == 1).all() and (lnb1 == 0).all() and (lng2 == 1).all()
        and (lnb2 == 0).all() and (ba == 0).all() and (bp == 0).all()
        and (bf_ == 0).all() and (bf2 == 0).all()
    )

    # Key compaction: permute tokens per batch so unmasked keys come first.
    mask01 = (mask != 0)
    counts = mask01.sum(axis=1)
    compact = bool(counts.max() <= COMPACT_KT * 128)
    kt_chunks = COMPACT_KT if compact else TT

    perms = []
    for b in range(N_CORES):
        perm = np.argsort(~mask01[b], kind="stable")  # unmasked first
        perms.append(perm)

    nc = _get_nc(trivial, kt_chunks)

    if trivial:
        cvec, dvec, w12_bf, dout = _mlp_linearization(
            f32(inputs["W_fc"]), f32(inputs["W_fc2"])
        )
        common = {
            "w_attn": _arr(_f8(f32(inputs["W_attn"]), 16.0)),
            "w_proj": _arr(_f8(f32(inputs["W_proj"]), 16.0)),
            "w_fc": _arr(_f8(f32(inputs["W_fc"]), 16.0)),
            "w_fc2": _arr(_f8(f32(inputs["W_fc2"]), 32.0)),
            "w12": _arr(w12_bf),
            "cvec": _col(cvec),
            "dvec": _col(dvec),
            "dout": dout,
        }
    else:
        common = {
            "w_attn": bf(inputs["W_attn"]),
            "w_proj": bf(inputs["W_proj"]),
            "w_fc": bf(inputs["W_fc"]),
            "w_fc2": bf(inputs["W_fc2"]),
            "ln1_g": lng1, "ln1_b": lnb1, "ln2_g": lng2, "ln2_b": lnb2,
            "b_attn": ba, "b_proj": bp, "b_fc": bf_, "b_fc2": bf2,
        }
    in_maps = []
    for b in range(N_CORES):
        m = dict(common)
        m["x"] = np.ascontiguousarray(x[b][perms[b]])
        m01 = mask01[b][perms[b]].astype(np.float32)
        if trivial:
            m01 = np.ascontiguousarray(m01.reshape(TT, 128).T)
        m["mask01"] = np.ascontiguousarray(m01)
        in_maps.append(m)

    from concourse.bass_utils import run_bass_kernel_spmd

    LAST_IN_MAPS = in_maps
    res = run_bass_kernel_spmd(nc, in_maps, core_ids=list(range(N_CORES)), trace=TRACE)
    LAST_RESULTS = res
    out = np.empty((N_CORES, T, C), np.float32)
    for b in range(N_CORES):
        out[b, perms[b]] = np.asarray(res.results[b]["out"])
    return out



# revision 35
# speedup vs baseline: 1.0345x; 1.0345x over previous
"""Trainium2 Bass kernel for one GPT-style transformer block.

Problem: B=8, T=1024, C=768, NH=12 heads (HD=64), pre-LN attention + MLP,
key-padding mask, tanh-gelu.  Sharding: data-parallel over batch — each of
the 8 NeuronCores processes one batch element end-to-end (no collectives).

Trivial path (unit LN gains / zero biases — what setup_inputs() generates)
uses fp8e4m3 DoubleRow matmuls (2 contraction chunks per PE pass) for the
QKV, attention-proj and fc2 matmuls, plus a linearized-gelu decomposition
that keeps the error in budget:

  - h1 = LN1(x) stored fp8 as 8*h (scale folded into rstd via eps/64 trick);
    W_attn quantized host-side to fp8(16*W).  q,k stored bf16 at 128x scale
    (exp scale becomes 2^-17); v stored in vext at 128x with a 16*mask ones
    column so o = pav * (1/pav[64]) comes out as 8*vbar, which is exactly the
    fp8 scale wanted for oT.  proj: fp8(16*W_proj) DoubleRow; the 1/128
    descale is fused into the residual add via scalar_tensor_tensor.
  - MLP: gelu(z) = c*z + d + r(z), with per-column least-squares (c, d)
    computed host-side from ||W_fc[:,j]|| via Gauss-Hermite.  The linear
    part goes through W12 = 32*(W_fc*diag(c))@W_fc2 in bf16 (768x768, cheap,
    accurate, bypasses fc1 error); d@W_fc2 is a host constant added to x;
    only the small residual r = gelu(z) - (c*z+d) is quantized fp8 and hits
    W_fc2 (fp8, 32x) with DoubleRow.  fc1 itself stays bf16 (its fp8 error
    would blow the tolerance).  Both parts accumulate into one PSUM tile;
    the 1/32 descale fuses into the final residual add.
  - Key compaction (unchanged): tokens permuted so unmasked keys come
    first; <=640 unmasked keys -> 5 of 8 key chunks processed.

General path (nonzero biases/gains) keeps the plain bf16 implementation.
"""

import numpy as np
import ml_dtypes

import concourse.bass as bass
import concourse.mybir as mybir
import concourse.tile as tile
from concourse import bacc
from concourse.bass import ds, ts
from concourse.masks import make_identity

F32 = mybir.dt.float32
BF16 = mybir.dt.bfloat16
F8 = mybir.dt.float8e4
AF = mybir.ActivationFunctionType
ALU = mybir.AluOpType
DR = mybir.MatmulPerfMode.DoubleRow

T, C, NH, HD = 1024, 768, 12, 64
TT = T // 128          # 8 token tiles
CC = C // 128          # 6 feature chunks
FC = (4 * C) // 128    # 24 ffn-hidden chunks
N_CORES = 8
EPS = 1e-5
EXP_SCALE = 0.125 / 16384.0   # 2^-17: q,k carry 128x scale each


def _bcast(ap_1d: bass.AP, p: int = 128) -> bass.AP:
    """Broadcast a 1-D DRAM AP across p partitions (zero partition stride)."""
    return bass.AP(tensor=ap_1d.tensor, offset=ap_1d.offset, ap=[[0, p]] + ap_1d.ap)


def build_bass(repeat: int = 1, trivial: bool = True, kt_chunks: int = 8) -> bass.Bass:
    if trivial:
        return _build_trivial_fp8(repeat, kt_chunks)
    return _build_general(repeat, kt_chunks)


# ====================================================================
# trivial path: fp8 DoubleRow + linearized MLP
# ====================================================================
def _build_trivial_fp8(repeat: int, kt_chunks: int) -> bass.Bass:
    KT = kt_chunks
    nc = bacc.Bacc(None)

    # weights arrive host-pre-arranged in SBUF layout [128, chunk, cols]:
    # every DMA below is a plain 2D contiguous copy on the hardware DGE
    # (no gpsimd descriptor generation, no rearrange)
    x_d = nc.dram_tensor("x", [T, C], F32, kind="ExternalInput")
    mask_d = nc.dram_tensor("mask01", [128, TT], F32, kind="ExternalInput")
    wattn_d = nc.dram_tensor("w_attn", [128, CC, 3 * C], F8, kind="ExternalInput")
    wproj_d = nc.dram_tensor("w_proj", [128, CC, C], F8, kind="ExternalInput")
    wfc_d = nc.dram_tensor("w_fc", [128, CC, 4 * C], F8, kind="ExternalInput")
    wfc2_d = nc.dram_tensor("w_fc2", [128, FC, C], F8, kind="ExternalInput")
    w12_d = nc.dram_tensor("w12", [128, CC, C], BF16, kind="ExternalInput")
    cvec_d = nc.dram_tensor("cvec", [128, FC], F32, kind="ExternalInput")
    dvec_d = nc.dram_tensor("dvec", [128, FC], F32, kind="ExternalInput")
    dout_d = nc.dram_tensor("dout", [C], F32, kind="ExternalInput")
    out_d = nc.dram_tensor("out", [T, C], F32, kind="ExternalOutput")

    with tile.TileContext(nc) as tc:
        from contextlib import ExitStack

        with ExitStack() as ctx:
            consts = ctx.enter_context(tc.tile_pool(name="consts", bufs=1))
            xpool = ctx.enter_context(tc.tile_pool(name="xpool", bufs=1))
            htmp_pool = ctx.enter_context(tc.tile_pool(name="htmp", bufs=3))
            stat_pool = ctx.enter_context(tc.tile_pool(name="stats", bufs=6))
            hT2_pool = ctx.enter_context(tc.tile_pool(name="hT2", bufs=1))
            wproj_pool = ctx.enter_context(tc.tile_pool(name="wproj", bufs=1))
            wfc_pool = ctx.enter_context(tc.tile_pool(name="wfc", bufs=1))
            ps_mm = ctx.enter_context(tc.tile_pool(name="ps_mm", bufs=2, space="PSUM"))
            ps_s = ctx.enter_context(tc.tile_pool(name="ps_s", bufs=3, space="PSUM"))
            ps_tr = ctx.enter_context(tc.tile_pool(name="ps_tr", bufs=2, space="PSUM"))

            # ---------------- constants ----------------
            ident = consts.tile([128, 128], BF16, name="ident")
            make_identity(nc, ident)
            mask_col = consts.tile([128, TT], F32, name="mask_col")
            nc.gpsimd.dma_start(out=mask_col, in_=mask_d[:, :])
            # LN1 feeds fp8 h at 8x: rstd' = 8/sigma via sqrt((var+eps)/64)
            eps64_t = consts.tile([128, 1], F32, name="eps64_t")
            nc.vector.memset(eps64_t, EPS / 64.0)
            eps_t = consts.tile([128, 1], F32, name="eps_t")
            nc.vector.memset(eps_t, EPS)
            c_col = consts.tile([128, FC], F32, name="c_col")
            nc.gpsimd.dma_start(out=c_col, in_=cvec_d[:, :])
            d_col = consts.tile([128, FC], F32, name="d_col")
            nc.gpsimd.dma_start(out=d_col, in_=dvec_d[:, :])
            dout_bc = consts.tile([128, C], F32, name="dout_bc")
            nc.gpsimd.dma_start(out=dout_bc, in_=_bcast(dout_d[:]))

            def layer_norm_to_hT(x_slice, hT, t, s8, hT_f8=None):
                """LN over C of one token tile -> bf16 -> PE transpose ->
                hT[:, c, t*128:...] (hT dtype may be fp8; copy converts).
                s8: emit 8*normalized (for fp8 targets).  hT_f8: optional
                fp8 copy derived from hT by the (otherwise idle) gpsimd
                engine, keeping the psum drain on ACT only."""
                stats = stat_pool.tile([128, 2, 6], F32, name="stats", tag="lnstats")
                for i in range(2):
                    nc.vector.bn_stats(out=stats[:, i, :], in_=x_slice[:, ts(i, 384)])
                mv = stat_pool.tile([128, 2], F32, name="mv", tag="lnmv")
                nc.vector.bn_aggr(out=mv, in_=stats)
                rstd = stat_pool.tile([128, 1], F32, name="rstd", tag="rstd")
                if s8:
                    nc.scalar.activation(
                        out=rstd, in_=mv[:, 1:2], func=AF.Sqrt,
                        bias=eps64_t[:, 0:1], scale=1.0 / 64.0,
                    )
                else:
                    nc.scalar.activation(
                        out=rstd, in_=mv[:, 1:2], func=AF.Sqrt, bias=eps_t[:, 0:1]
                    )
                nc.vector.reciprocal(rstd, rstd)
                hbf = htmp_pool.tile([128, C], BF16, name="hbf", tag="hbf")
                nc.vector.tensor_scalar(
                    out=hbf, in0=x_slice, scalar1=mv[:, 0:1], scalar2=rstd,
                    op0=ALU.subtract, op1=ALU.mult,
                )
                for c in range(CC):
                    ptr = ps_tr.tile([128, 128], BF16, name="ptr", tag="tr")
                    nc.tensor.transpose(ptr, hbf[:, ts(c, 128)], ident)
                    nc.scalar.copy(hT[:, c, ts(t, 128)], ptr)
                    if hT_f8 is not None:
                        nc.gpsimd.tensor_copy(hT_f8[:, c, ts(t, 128)], hT[:, c, ts(t, 128)])

            # ================= one full block (repeatable) =================
            for _rep in range(repeat):
              x_t = xpool.tile([128, TT, C], F32, name="x_t", tag="x_t")

              # ---- attention scope ----
              with ExitStack() as actx:
                wattn_pool = actx.enter_context(tc.tile_pool(name="wattn", bufs=1))
                hT_pool = actx.enter_context(tc.tile_pool(name="hT1", bufs=1))
                qkT_pool = actx.enter_context(tc.tile_pool(name="qkT", bufs=1))
                vext_pool = actx.enter_context(tc.tile_pool(name="vext", bufs=1))
                sT_pool = actx.enter_context(tc.tile_pool(name="sT", bufs=4))
                opool = actx.enter_context(tc.tile_pool(name="opool", bufs=1))

                # DMA issue order is engine-processing order: x0, x1 first
                # (LN critical path), then wattn pair 0 (first matmul
                # operand), then the rest interleaved.
                wattn_sb = wattn_pool.tile([128, CC, 3 * C], F8, name="wattn_sb")

                def wattn_dma(cp):
                    nc.sync.dma_start(
                        out=wattn_sb[:, 2 * cp : 2 * cp + 2, :],
                        in_=wattn_d[:, 2 * cp : 2 * cp + 2, :],
                    )

                for t in range(2):
                    # column-halves so bn_stats(i=0) starts half a DMA early
                    for i in range(2):
                        nc.sync.dma_start(
                            out=x_t[:, t, ts(i, 384)],
                            in_=x_d[ts(t, 128), ts(i, 384)],
                        )
                wattn_dma(0)
                for t in range(2, 4):
                    nc.sync.dma_start(out=x_t[:, t, :], in_=x_d[ts(t, 128), :])
                wattn_dma(1)
                for t in range(4, 6):
                    nc.sync.dma_start(out=x_t[:, t, :], in_=x_d[ts(t, 128), :])
                wattn_dma(2)
                for t in range(6, TT):
                    nc.sync.dma_start(out=x_t[:, t, :], in_=x_d[ts(t, 128), :])
                wproj_sb = wproj_pool.tile([128, CC, C], F8, name="wproj_sb")
                nc.sync.dma_start(out=wproj_sb, in_=wproj_d[:, :, :])
                # wfc lives in an outer pool (no address overlap with the
                # attention pools) so its DMA streams during attention and
                # fc1 never waits on it at the MLP seam.
                wfc_sb = wfc_pool.tile([128, CC, 4 * C], F8, name="wfc_sb")
                nc.sync.dma_start(out=wfc_sb, in_=wfc_d[:, :, :])

                h1T = hT_pool.tile([128, CC, T], F8, name="h1T", tag="hT")
                qT = qkT_pool.tile([128, CC, T], BF16, name="qT", tag="qT")
                kT = qkT_pool.tile([128, CC, KT * 128], BF16, name="kT", tag="kT")
                vext = vext_pool.tile([128, NH, KT, HD + 1], BF16, name="vext")

                # q^T / k^T feature-major via fp8 DoubleRow (3 c-pair passes);
                # 256-col phases so matmuls start after just 2 LN'd tiles
                def qk_phase(nqi, interleave=False):
                    for m in range(12):
                        dest = qT if m < 6 else kT
                        nlim = T if m < 6 else KT * 128
                        n0 = nqi * 256
                        nsz = min(256, nlim - n0)
                        if nsz <= 0:
                            if interleave:
                                drain(1)
                            continue
                        pq = ps_mm.tile([128, nsz], F32, name="pq", tag="mm")
                        for cp in range(CC // 2):
                            nc.tensor.matmul(
                                pq,
                                lhsT=wattn_sb[:, 2 * cp : 2 * cp + 2, ts(m, 128)],
                                rhs=h1T[:, 2 * cp : 2 * cp + 2, ds(n0, nsz)],
                                start=(cp == 0),
                                stop=(cp == CC // 2 - 1),
                                perf_mode=DR,
                            )
                        nc.vector.tensor_copy(dest[:, m % 6, ds(n0, nsz)], pq)
                        if interleave:
                            drain(1)

                # LN t0-t3 issued first so the DVE FIFO isn't blocked behind
                # qk psum-copies; qk_phase(p) only needs tiles 2p,2p+1
                for t in range(4):
                    layer_norm_to_hT(x_t[:, t, :], h1T, t, s8=True)
                # ---- software-pipelined scores / AV / proj ----
                # The exp pipeline (ACT) paces scores: each (kt, nq)
                # sub-chunk is 4 concurrent K=64 matmuls (disjoint PE
                # row-groups via auto tile_position) + 2 exps.  Sub-chunks
                # are queued as "fillers" and drained one at a time between
                # AV chains / v-part groups / qk blocks so the in-order PE
                # stream always has independent work while ACT catches up.
                sT_tiles = {}
                fillers = []

                def drain(n=1):
                    for _ in range(min(n, len(fillers))):
                        fillers.pop(0)()

                def pair_subchunk(hc, kt, nq):
                    pss = []
                    for hr in range(2):
                        ps = ps_s.tile([128, 512], F32, name="pss", tag="ss")
                        pss.append(ps)
                        nc.tensor.matmul(
                            ps,
                            lhsT=kT[ds(hr * 64, 64), hc, ts(kt, 128)],
                            rhs=qT[ds(hr * 64, 64), hc, ts(nq, 512)],
                            start=True,
                            stop=True,
                        )
                    for hr in range(2):
                        nc.scalar.activation(
                            out=sT_tiles[2 * hc + hr][:, kt, ts(nq, 512)],
                            in_=pss[hr],
                            func=AF.Exp,
                            scale=EXP_SCALE,
                        )

                def queue_pair(hc, nqs=(0, 1)):
                    for hr in range(2):
                        if 2 * hc + hr not in sT_tiles:
                            sT_tiles[2 * hc + hr] = sT_pool.tile(
                                [128, KT, T], BF16, name="sT", tag="sT"
                            )
                    for nq in nqs:
                        for kt in range(KT):
                            fillers.append(
                                lambda hc=hc, kt=kt, nq=nq: pair_subchunk(hc, kt, nq)
                            )

                qk_phase(0)
                for t in range(4, 6):
                    layer_norm_to_hT(x_t[:, t, :], h1T, t, s8=True)
                qk_phase(1)
                for t in range(6, TT):
                    layer_norm_to_hT(x_t[:, t, :], h1T, t, s8=True)
                qk_phase(2)
                # nq=0 sub-chunks only need qT[:, :, 0:512] (phases 0-1) and
                # kT fully (phase 2) -> interleave them into phase 3
                queue_pair(0, nqs=(0,))
                queue_pair(1, nqs=(0,))
                qk_phase(3, interleave=True)
                queue_pair(0, nqs=(1,))
                queue_pair(1, nqs=(1,))

                # ones column = 16*mask: AV denominator = 16*sum(p), so
                # o_t = pav * (1/pav[64]) = 8 * vbar  (v stored at 128x)
                nc.vector.memset(vext[:, :, :, HD : HD + 1], 16.0)
                for t in range(KT):
                    nc.vector.tensor_scalar_mul(
                        out=vext[:, :, t, HD : HD + 1],
                        in0=vext[:, :, t, HD : HD + 1],
                        scalar1=mask_col[:, t : t + 1],
                    )

                # v token-major at 128x scale, masked, scattered per head
                # (256-wide chunks: DoubleRow moving limit is 2*256 elems)
                for t in range(KT):
                    for n0 in range(0, C, 256):
                        pv = ps_mm.tile([128, 256], F32, name="pv", tag="mm")
                        for cp in range(CC // 2):
                            nc.tensor.matmul(
                                pv,
                                lhsT=h1T[:, 2 * cp : 2 * cp + 2, ts(t, 128)],
                                rhs=wattn_sb[:, 2 * cp : 2 * cp + 2, ds(2 * C + n0, 256)],
                                start=(cp == 0),
                                stop=(cp == CC // 2 - 1),
                                perf_mode=DR,
                            )
                        h0, h1 = n0 // HD, (n0 + 256) // HD
                        pv_h = pv.rearrange("p (h d) -> p h d", d=HD)
                        nc.vector.tensor_scalar_mul(
                            out=vext[:, h0:h1, t, 0:HD], in0=pv_h,
                            scalar1=mask_col[:, t : t + 1],
                        )
                        drain(1)

                o_t = opool.tile([128, TT, C], BF16, name="o_t", tag="op")
                oT = qkT_pool.tile([128, CC, T], F8, name="oT", tag="oT")

                def proj_tile(cp, t):
                    # partial proj contribution of oT c-pair cp for tile t,
                    # folded into x (the 1/128 descale distributes over the
                    # sum).  Queued as a filler once chunk cp of oT exists.
                    for n0 in range(0, C, 256):
                        pp = ps_mm.tile([128, 256], F32, name="pp", tag="mm")
                        nc.tensor.matmul(
                            pp,
                            lhsT=oT[:, 2 * cp : 2 * cp + 2, ts(t, 128)],
                            rhs=wproj_sb[:, 2 * cp : 2 * cp + 2, ds(n0, 256)],
                            start=True,
                            stop=True,
                            perf_mode=DR,
                        )
                        nc.vector.scalar_tensor_tensor(
                            out=x_t[:, t, ds(n0, 256)], in0=pp,
                            scalar=1.0 / 128.0, in1=x_t[:, t, ds(n0, 256)],
                            op0=ALU.mult, op1=ALU.add,
                        )

                def queue_proj(cp):
                    for t in range(TT):
                        fillers.append(lambda cp=cp, t=t: proj_tile(cp, t))

                # AV: 4 token tiles accumulate into one psum bank, then one
                # batched reciprocal + one broadcast multiply normalize all
                # four (instead of per-tile recip+mul DVE pairs).
                for h in range(NH):
                    sT = sT_tiles.pop(h)
                    for half in range(2):
                        pav8 = ps_mm.tile(
                            [128, 4, 128], F32, name="pav8", tag="av", bufs=1
                        )
                        for i in range(4):
                            tq = half * 4 + i
                            for kt in range(KT):
                                nc.tensor.matmul(
                                    pav8[:, i, 0 : HD + 1],
                                    lhsT=sT[:, kt, ts(tq, 128)],
                                    rhs=vext[:, h, kt, :],
                                    start=(kt == 0),
                                    stop=(kt == KT - 1),
                                )
                            drain(1)
                        rec4 = stat_pool.tile([128, 4], F32, name="rec4", tag="rec")
                        nc.vector.reciprocal(rec4, pav8[:, :, HD : HD + 1])
                        nc.vector.tensor_tensor(
                            out=o_t[:, ds(half * 4, 4), ts(h, HD)],
                            in0=pav8[:, :, 0:HD],
                            in1=rec4[:, :].unsqueeze(2).broadcast_to([128, 4, HD]),
                            op=ALU.mult,
                        )
                        if h % 2 == 1:
                            # both heads of chunk h//2 done for these tiles:
                            # transpose now so proj can chase the AV sweep
                            c = h // 2
                            for i in range(4):
                                tq = half * 4 + i
                                ptr = ps_tr.tile([128, 128], BF16, name="ptr2", tag="tr")
                                nc.tensor.transpose(ptr, o_t[:, tq, ts(c, 128)], ident)
                                nc.vector.tensor_copy(oT[:, c, ts(tq, 128)], ptr)
                        drain(1)
                    if h % 2 == 1:
                        if h + 3 < NH:
                            queue_pair((h + 3) // 2)
                        if h % 4 == 3:
                            # oT c-pair h//4 complete (heads h-3..h) -> its
                            # proj contribution can chase the AV sweep
                            queue_proj(h // 4)

                # tail: the last proj chunk drains 1:1 with LN2 tiles.  LN2
                # emits 8*normalized twice: bf16 via ACT (feeds the exact
                # W12 linear path) and an fp8 gpsimd copy (feeds fc1 DR).
                assert len(fillers) == TT, f"filler backlog {len(fillers)}"
                h2f = hT2_pool.tile([128, CC, T], F8, name="h2f", tag="hT2f")
                h2b = hT2_pool.tile([128, CC, T], BF16, name="h2b", tag="hT2b")
                for t in range(TT):
                    drain(2)
                    layer_norm_to_hT(x_t[:, t, :], h2b, t, s8=True, hT_f8=h2f)
                assert not fillers

              # ---- MLP scope ----
              with ExitStack() as mctx:
                wfc2_pool = mctx.enter_context(tc.tile_pool(name="wfc2", bufs=1))
                w12_pool = mctx.enter_context(tc.tile_pool(name="w12", bufs=1))
                rT_pool = mctx.enter_context(tc.tile_pool(name="rT", bufs=2))
                gz_pool = mctx.enter_context(tc.tile_pool(name="gz", bufs=6))
                outsb_pool = mctx.enter_context(tc.tile_pool(name="outsb", bufs=2))

                wfc2_sb = wfc2_pool.tile([128, FC, C], F8, name="wfc2_sb")
                nc.sync.dma_start(out=wfc2_sb, in_=wfc2_d[:, :, :])
                w12_sb = w12_pool.tile([128, CC, C], BF16, name="w12_sb")
                nc.sync.dma_start(out=w12_sb, in_=w12_d[:, :, :])

                # x += dout (the d@W_fc2 constant) once per tile; x_t was
                # already consumed by LN2 above so this is safe
                for t in range(TT):
                    nc.vector.tensor_add(x_t[:, t, :], x_t[:, t, :], dout_bc)

                def fc1_m(half, rT, m):
                    # fc1 in fp8 DoubleRow: psum = 128*z (8x h2 * 16x W_fc);
                    # the 1/128 descale folds into the gelu scale and the
                    # host-prescaled c vector.  N=256 chunks (DR moving
                    # limit), sequential accumulation groups per chunk.
                    pf = ps_s.tile([128, 512], F32, name="pf", tag="ss")
                    for ni in range(2):
                        for cp in range(CC // 2):
                            nc.tensor.matmul(
                                pf[:, ts(ni, 256)],
                                lhsT=wfc_sb[:, 2 * cp : 2 * cp + 2, ts(m, 128)],
                                rhs=h2f[:, 2 * cp : 2 * cp + 2,
                                        ds(half * 512 + ni * 256, 256)],
                                start=(cp == 0),
                                stop=(cp == CC // 2 - 1),
                                perf_mode=DR,
                            )
                    g = gz_pool.tile([128, 512], BF16, name="g", tag="g")
                    nc.scalar.activation(
                        out=g, in_=pf, func=AF.Gelu_apprx_tanh, scale=1.0 / 128.0
                    )
                    zc = gz_pool.tile([128, 512], BF16, name="zc", tag="zc")
                    nc.vector.tensor_scalar(
                        out=zc, in0=pf, scalar1=c_col[:, m : m + 1],
                        scalar2=d_col[:, m : m + 1], op0=ALU.mult, op1=ALU.add,
                    )
                    # SBUF-only subtract, alternated between the idle
                    # gpsimd engine and DVE so neither paces the fc1 loop
                    eng = nc.gpsimd if m % 2 == 0 else nc.vector
                    eng.tensor_sub(rT[:, m, :], g, zc)

                def fc2_tile(half, rT, i):
                    t = half * 4 + i
                    outsb = outsb_pool.tile([128, C], F32, name="outsb", tag="outsb")
                    for n0 in range(0, C, 256):
                        pf2 = ps_s.tile([128, 256], F32, name="pf2", tag="ss")
                        # linear part: 8*h2 @ 4*W12 (bf16, 32x total)
                        for c in range(CC):
                            nc.tensor.matmul(
                                pf2,
                                lhsT=h2b[:, c, ts(t, 128)],
                                rhs=w12_sb[:, c, ds(n0, 256)],
                                start=(c == 0),
                                stop=False,
                            )
                        # residual part: r @ W_fc2 (fp8 DR, 32x)
                        for mp in range(FC // 2):
                            nc.tensor.matmul(
                                pf2,
                                lhsT=rT[:, 2 * mp : 2 * mp + 2, ts(i, 128)],
                                rhs=wfc2_sb[:, 2 * mp : 2 * mp + 2, ds(n0, 256)],
                                start=False,
                                stop=(mp == FC // 2 - 1),
                                perf_mode=DR,
                            )
                        nc.vector.scalar_tensor_tensor(
                            out=outsb[:, ds(n0, 256)], in0=pf2,
                            scalar=1.0 / 32.0, in1=x_t[:, t, ds(n0, 256)],
                            op0=ALU.mult, op1=ALU.add,
                        )
                        nc.sync.dma_start(
                            out=out_d[ts(t, 128), ds(n0, 256)],
                            in_=outsb[:, ds(n0, 256)],
                        )

                # half-0 fc1, then half-1 fc1 interleaved with half-0 fc2
                # tiles so PE never waits on the Pool r-subtract pipeline
                rT0 = rT_pool.tile([128, FC, 512], F8, name="rT0", tag="rT")
                for m in range(FC):
                    fc1_m(0, rT0, m)
                rT1 = rT_pool.tile([128, FC, 512], F8, name="rT1", tag="rT")
                for i in range(4):
                    for m in range(6 * i, 6 * i + 6):
                        fc1_m(1, rT1, m)
                    fc2_tile(0, rT0, i)
                for i in range(4):
                    fc2_tile(1, rT1, i)

    return nc


# ====================================================================
# general path: plain bf16 (biases/gains applied) — unchanged baseline
# ====================================================================
def _build_general(repeat: int, kt_chunks: int) -> bass.Bass:
    KT = kt_chunks
    nc = bacc.Bacc(None)

    x_d = nc.dram_tensor("x", [T, C], F32, kind="ExternalInput")
    mask_d = nc.dram_tensor("mask01", [T], F32, kind="ExternalInput")
    wattn_d = nc.dram_tensor("w_attn", [C, 3 * C], BF16, kind="ExternalInput")
    wproj_d = nc.dram_tensor("w_proj", [C, C], BF16, kind="ExternalInput")
    wfc_d = nc.dram_tensor("w_fc", [C, 4 * C], BF16, kind="ExternalInput")
    wfc2_d = nc.dram_tensor("w_fc2", [4 * C, C], BF16, kind="ExternalInput")
    ln1g_d = nc.dram_tensor("ln1_g", [C], F32, kind="ExternalInput")
    ln1b_d = nc.dram_tensor("ln1_b", [C], F32, kind="ExternalInput")
    ln2g_d = nc.dram_tensor("ln2_g", [C], F32, kind="ExternalInput")
    ln2b_d = nc.dram_tensor("ln2_b", [C], F32, kind="ExternalInput")
    battn_d = nc.dram_tensor("b_attn", [3 * C], F32, kind="ExternalInput")
    bproj_d = nc.dram_tensor("b_proj", [C], F32, kind="ExternalInput")
    bfc_d = nc.dram_tensor("b_fc", [4 * C], F32, kind="ExternalInput")
    bfc2_d = nc.dram_tensor("b_fc2", [C], F32, kind="ExternalInput")
    out_d = nc.dram_tensor("out", [T, C], F32, kind="ExternalOutput")

    with tile.TileContext(nc) as tc:
        from contextlib import ExitStack

        with ExitStack() as ctx:
            consts = ctx.enter_context(tc.tile_pool(name="consts", bufs=1))
            xpool = ctx.enter_context(tc.tile_pool(name="xpool", bufs=1))
            htmp_pool = ctx.enter_context(tc.tile_pool(name="htmp", bufs=3))
            stat_pool = ctx.enter_context(tc.tile_pool(name="stats", bufs=6))
            hT2_pool = ctx.enter_context(tc.tile_pool(name="hT2", bufs=1))
            wproj_pool = ctx.enter_context(tc.tile_pool(name="wproj", bufs=1))
            ps_mm = ctx.enter_context(tc.tile_pool(name="ps_mm", bufs=2, space="PSUM"))
            ps_s = ctx.enter_context(tc.tile_pool(name="ps_s", bufs=2, space="PSUM"))
            ps_tr = ctx.enter_context(tc.tile_pool(name="ps_tr", bufs=2, space="PSUM"))

            ident = consts.tile([128, 128], BF16, name="ident")
            make_identity(nc, ident)
            mask_col = consts.tile([128, TT], F32, name="mask_col")
            nc.gpsimd.dma_start(out=mask_col, in_=mask_d[:].rearrange("(t p) -> p t", p=128))
            eps_t = consts.tile([128, 1], F32, name="eps_t")
            nc.vector.memset(eps_t, EPS)

            g1_bc = consts.tile([128, C], F32, name="g1_bc")
            b1_bc = consts.tile([128, C], F32, name="b1_bc")
            g2_bc = consts.tile([128, C], F32, name="g2_bc")
            b2_bc = consts.tile([128, C], F32, name="b2_bc")
            battnv_bc = consts.tile([128, C], F32, name="battnv_bc")
            bproj_bc = consts.tile([128, C], F32, name="bproj_bc")
            bfc2_bc = consts.tile([128, C], F32, name="bfc2_bc")
            nc.gpsimd.dma_start(out=g1_bc, in_=_bcast(ln1g_d[:]))
            nc.gpsimd.dma_start(out=b1_bc, in_=_bcast(ln1b_d[:]))
            nc.gpsimd.dma_start(out=g2_bc, in_=_bcast(ln2g_d[:]))
            nc.gpsimd.dma_start(out=b2_bc, in_=_bcast(ln2b_d[:]))
            nc.gpsimd.dma_start(out=battnv_bc, in_=_bcast(battn_d[ds(2 * C, C)]))
            nc.gpsimd.dma_start(out=bproj_bc, in_=_bcast(bproj_d[:]))
            nc.gpsimd.dma_start(out=bfc2_bc, in_=_bcast(bfc2_d[:]))
            battn_qk = consts.tile([128, 12], F32, name="battn_qk")
            nc.gpsimd.dma_start(
                out=battn_qk,
                in_=battn_d[ds(0, 2 * C)].rearrange("(m p) -> p m", p=128),
            )
            bfc_col = consts.tile([128, FC], F32, name="bfc_col")
            nc.gpsimd.dma_start(
                out=bfc_col, in_=bfc_d[:].rearrange("(m p) -> p m", p=128)
            )

            def layer_norm_to_hT(x_slice, g_bc, b_bc, hT, t):
                stats = stat_pool.tile([128, 2, 6], F32, name="stats", tag="lnstats")
                for i in range(2):
                    nc.vector.bn_stats(out=stats[:, i, :], in_=x_slice[:, ts(i, 384)])
                mv = stat_pool.tile([128, 2], F32, name="mv", tag="lnmv")
                nc.vector.bn_aggr(out=mv, in_=stats)
                rstd = stat_pool.tile([128, 1], F32, name="rstd", tag="rstd")
                nc.scalar.activation(out=rstd, in_=mv[:, 1:2], func=AF.Sqrt, bias=eps_t[:, 0:1])
                nc.vector.reciprocal(rstd, rstd)
                hbf = htmp_pool.tile([128, C], BF16, name="hbf", tag="hbf")
                htmp = htmp_pool.tile([128, C], F32, name="htmp", tag="htmp")
                nc.vector.tensor_scalar(
                    out=htmp, in0=x_slice, scalar1=mv[:, 0:1], scalar2=rstd,
                    op0=ALU.subtract, op1=ALU.mult,
                )
                nc.vector.tensor_mul(htmp, htmp, g_bc)
                nc.vector.tensor_add(hbf, htmp, b_bc)
                for c in range(CC):
                    ptr = ps_tr.tile([128, 128], BF16, name="ptr", tag="tr")
                    nc.tensor.transpose(ptr, hbf[:, ts(c, 128)], ident)
                    nc.scalar.copy(hT[:, c, ts(t, 128)], ptr)

            for _rep in range(repeat):
              x_t = xpool.tile([128, TT, C], F32, name="x_t", tag="x_t")
              for t in range(TT):
                  nc.sync.dma_start(out=x_t[:, t, :], in_=x_d[ts(t, 128), :])

              with ExitStack() as actx:
                wattn_pool = actx.enter_context(tc.tile_pool(name="wattn", bufs=1))
                hT_pool = actx.enter_context(tc.tile_pool(name="hT1", bufs=1))
                qkT_pool = actx.enter_context(tc.tile_pool(name="qkT", bufs=1))
                vext_pool = actx.enter_context(tc.tile_pool(name="vext", bufs=1))
                sT_pool = actx.enter_context(tc.tile_pool(name="sT", bufs=3))
                opool = actx.enter_context(tc.tile_pool(name="opool", bufs=1))

                wattn_sb = wattn_pool.tile([128, CC, 3 * C], BF16, name="wattn_sb")
                nc.gpsimd.dma_start(
                    out=wattn_sb, in_=wattn_d[:, :].rearrange("(c p) n -> p c n", p=128)
                )
                wproj_sb = wproj_pool.tile([128, CC, C], BF16, name="wproj_sb")
                nc.gpsimd.dma_start(
                    out=wproj_sb, in_=wproj_d[:, :].rearrange("(c p) n -> p c n", p=128)
                )

                h1T = hT_pool.tile([128, CC, T], BF16, name="h1T", tag="hT")
                qT = qkT_pool.tile([128, CC, T], BF16, name="qT", tag="qT")
                kT = qkT_pool.tile([128, CC, KT * 128], BF16, name="kT", tag="kT")
                vext = vext_pool.tile([128, NH, KT, HD + 1], BF16, name="vext")
                nc.vector.memset(vext[:, :, :, HD : HD + 1], 1.0)

                def qk_phase(nqi):
                    for m in range(12):
                        dest = qT if m < 6 else kT
                        nlim = T if m < 6 else KT * 128
                        n0 = nqi * 512
                        nsz = min(512, nlim - n0)
                        if nsz <= 0:
                            continue
                        pq = ps_mm.tile([128, nsz], F32, name="pq", tag="mm")
                        for c in range(CC):
                            nc.tensor.matmul(
                                pq,
                                lhsT=wattn_sb[:, c, ts(m, 128)],
                                rhs=h1T[:, c, ds(n0, nsz)],
                                start=(c == 0),
                                stop=(c == CC - 1),
                            )
                        nc.scalar.activation(
                            out=dest[:, m % 6, ds(n0, nsz)], in_=pq,
                            func=AF.Identity, bias=battn_qk[:, m : m + 1],
                        )

                for t in range(4):
                    layer_norm_to_hT(x_t[:, t, :], g1_bc, b1_bc, h1T, t)
                qk_phase(0)
                for t in range(4, TT):
                    layer_norm_to_hT(x_t[:, t, :], g1_bc, b1_bc, h1T, t)
                qk_phase(1)

                sT_tiles = {}

                def emit_scores(h):
                    hc, hr = divmod(h, 2)
                    r0 = hr * 64
                    sT = sT_pool.tile([128, KT, T], BF16, name="sT", tag="sT")
                    sT_tiles[h] = sT
                    for kt in range(KT):
                        pss = ps_s.tile([128, 2, 512], F32, name="pss", tag="ss")
                        for nq in range(2):
                            nc.tensor.matmul(
                                pss[:, nq, :],
                                lhsT=kT[ds(r0, 64), hc, ts(kt, 128)],
                                rhs=qT[ds(r0, 64), hc, ts(nq, 512)],
                                start=True,
                                stop=True,
                            )
                        nc.scalar.activation(
                            out=sT[:, kt, :],
                            in_=pss.rearrange("p a b -> p (a b)"),
                            func=AF.Exp,
                            scale=0.125,
                        )

                emit_scores(0)
                emit_scores(1)
                emit_scores(2)

                for t in range(KT):
                    for n0, nsz in ((0, 512), (512, 256)):
                        pv = ps_mm.tile([128, nsz], F32, name="pv", tag="mm")
                        for c in range(CC):
                            nc.tensor.matmul(
                                pv,
                                lhsT=h1T[:, c, ts(t, 128)],
                                rhs=wattn_sb[:, c, ds(2 * C + n0, nsz)],
                                start=(c == 0),
                                stop=(c == CC - 1),
                            )
                        h0, h1 = n0 // HD, (n0 + nsz) // HD
                        pv_h = pv.rearrange("p (h d) -> p h d", d=HD)
                        nc.vector.tensor_add(
                            out=vext[:, h0:h1, t, 0:HD], in0=pv_h,
                            in1=battnv_bc[:, ds(n0, nsz)].rearrange(
                                "p (h d) -> p h d", d=HD
                            ),
                        )
                for t in range(KT):
                    nc.vector.tensor_scalar_mul(
                        out=vext[:, :, t, :], in0=vext[:, :, t, :],
                        scalar1=mask_col[:, t : t + 1],
                    )

                o_t = opool.tile([128, TT, C], BF16, name="o_t", tag="op")
                oT = qkT_pool.tile([128, CC, T], BF16, name="oT", tag="oT")
                for h in range(NH):
                    sT = sT_tiles.pop(h)
                    for tq in range(TT):
                        pav = ps_mm.tile([128, HD + 1], F32, name="pav", tag="mm")
                        for kt in range(KT):
                            nc.tensor.matmul(
                                pav,
                                lhsT=sT[:, kt, ts(tq, 128)],
                                rhs=vext[:, h, kt, :],
                                start=(kt == 0),
                                stop=(kt == KT - 1),
                            )
                        rec = stat_pool.tile([128, 1], F32, name="rec", tag="rec")
                        nc.vector.reciprocal(rec, pav[:, HD : HD + 1])
                        nc.vector.tensor_scalar_mul(
                            out=o_t[:, tq, ts(h, HD)], in0=pav[:, 0:HD], scalar1=rec
                        )
                    if h + 3 < NH:
                        emit_scores(h + 3)
                    if h % 2 == 1:
                        c = h // 2
                        for t in range(TT):
                            ptr = ps_tr.tile([128, 128], BF16, name="ptr2", tag="tr")
                            nc.tensor.transpose(ptr, o_t[:, t, ts(c, 128)], ident)
                            nc.vector.tensor_copy(oT[:, c, ts(t, 128)], ptr)

                h2T = hT2_pool.tile([128, CC, T], BF16, name="h2T", tag="hT2")
                for grp in range(2):
                    for t in range(grp * 4, grp * 4 + 4):
                        for n0, nsz in ((0, 512), (512, 256)):
                            pp = ps_mm.tile([128, nsz], F32, name="pp", tag="mm")
                            for c in range(CC):
                                nc.tensor.matmul(
                                    pp,
                                    lhsT=oT[:, c, ts(t, 128)],
                                    rhs=wproj_sb[:, c, ds(n0, nsz)],
                                    start=(c == 0),
                                    stop=(c == CC - 1),
                                )
                            nc.vector.tensor_add(pp, pp, bproj_bc[:, ds(n0, nsz)])
                            nc.vector.tensor_add(
                                x_t[:, t, ds(n0, nsz)], x_t[:, t, ds(n0, nsz)], pp
                            )
                    for t in range(grp * 4, grp * 4 + 4):
                        layer_norm_to_hT(x_t[:, t, :], g2_bc, b2_bc, h2T, t)

              with ExitStack() as mctx:
                wfc_pool = mctx.enter_context(tc.tile_pool(name="wfc", bufs=1))
                wfc2_pool = mctx.enter_context(tc.tile_pool(name="wfc2", bufs=1))
                aT_pool = mctx.enter_context(tc.tile_pool(name="aT", bufs=1))
                outsb_pool = mctx.enter_context(tc.tile_pool(name="outsb", bufs=2))

                wfc_sb = wfc_pool.tile([128, CC, 4 * C], BF16, name="wfc_sb")
                nc.gpsimd.dma_start(
                    out=wfc_sb, in_=wfc_d[:, :].rearrange("(c p) n -> p c n", p=128)
                )
                wfc2_sb = wfc2_pool.tile([128, FC, C], BF16, name="wfc2_sb")
                nc.gpsimd.dma_start(
                    out=wfc2_sb, in_=wfc2_d[:, :].rearrange("(m p) n -> p m n", p=128)
                )

                for half in range(2):
                    aT = aT_pool.tile([128, FC, 512], BF16, name="aT", tag="aT")
                    for m in range(FC):
                        pf = ps_mm.tile([128, 512], F32, name="pf", tag="mm")
                        for c in range(CC):
                            nc.tensor.matmul(
                                pf,
                                lhsT=wfc_sb[:, c, ts(m, 128)],
                                rhs=h2T[:, c, ds(half * 512, 512)],
                                start=(c == 0),
                                stop=(c == CC - 1),
                            )
                        nc.scalar.activation(
                            out=aT[:, m, :], in_=pf, func=AF.Gelu_apprx_tanh,
                            bias=bfc_col[:, m : m + 1],
                        )
                    for i in range(4):
                        t = half * 4 + i
                        outsb = outsb_pool.tile([128, C], F32, name="outsb", tag="outsb")
                        for n0, nsz in ((0, 512), (512, 256)):
                            pf2 = ps_mm.tile([128, nsz], F32, name="pf2", tag="mm")
                            for m in range(FC):
                                nc.tensor.matmul(
                                    pf2,
                                    lhsT=aT[:, m, ts(i, 128)],
                                    rhs=wfc2_sb[:, m, ds(n0, nsz)],
                                    start=(m == 0),
                                    stop=(m == FC - 1),
                                )
                            nc.vector.tensor_add(pf2, pf2, bfc2_bc[:, ds(n0, nsz)])
                            nc.vector.tensor_add(
                                outsb[:, ds(n0, nsz)], x_t[:, t, ds(n0, nsz)], pf2
                            )
                        nc.sync.dma_start(out=out_d[ts(t, 128), :], in_=outsb)

    return nc


_NC_CACHE = {}

COMPACT_KT = 5  # attention processes 5*128 = 640 keys; guarded in kernel()


def _get_nc(trivial: bool = True, kt_chunks: int = COMPACT_KT) -> bass.Bass:
    key = (trivial, kt_chunks)
    if key not in _NC_CACHE:
        nc = build_bass(trivial=trivial, kt_chunks=kt_chunks)
        nc.finalize()
        _NC_CACHE[key] = nc
    return _NC_CACHE[key]


TRACE = False
LAST_RESULTS = None
LAST_IN_MAPS = None


def _f8(a: np.ndarray, scale: float) -> np.ndarray:
    """Host-side quantize to TRN-compatible e4m3 (clip +-240) after scaling."""
    s = np.clip(a.astype(np.float32) * scale, -240.0, 240.0)
    return np.ascontiguousarray(s.astype(ml_dtypes.float8_e4m3))


def _arr(w: np.ndarray) -> np.ndarray:
    """[K, N] -> SBUF layout [128, K//128, N] (chunked along K)."""
    k, n = w.shape
    return np.ascontiguousarray(w.reshape(k // 128, 128, n).transpose(1, 0, 2))


def _col(v: np.ndarray) -> np.ndarray:
    """[M] -> [128, M//128] (per-partition column layout)."""
    return np.ascontiguousarray(v.reshape(-1, 128).T)


def _gelu64(a):
    return 0.5 * a * (1.0 + np.tanh(0.7978845608028654 * (a + 0.044715 * a**3)))


def _mlp_linearization(W_fc: np.ndarray, W_fc2: np.ndarray):
    """Per-column least-squares (c, d) for gelu under z_j ~ N(0, sigma_j),
    plus the 4x-scaled bf16 linear weight W12 (8x h2 * 4x W12 = 32x psum)
    and output constant dout.  c is pre-divided by 128 (fc1 psum carries
    128*z from the fp8 DoubleRow matmul)."""
    W1 = W_fc.astype(np.float64)
    W2 = W_fc2.astype(np.float64)
    gh_x, gh_w = np.polynomial.hermite_e.hermegauss(63)
    gh_w = gh_w / gh_w.sum()
    sigma = np.linalg.norm(W1, axis=0)
    sigma = np.maximum(sigma, 1e-12)
    zg = sigma[:, None] * gh_x[None, :]
    gz = _gelu64(zg)
    d_vec = (gz * gh_w[None, :]).sum(1)
    c_vec = (zg * gz * gh_w[None, :]).sum(1) / (sigma**2)
    W12 = (W1 * c_vec[None, :]) @ W2
    dout = d_vec @ W2
    return (
        np.ascontiguousarray((c_vec / 128.0).astype(np.float32)),
        np.ascontiguousarray(d_vec.astype(np.float32)),
        np.ascontiguousarray((4.0 * W12).astype(np.float32).astype(ml_dtypes.bfloat16)),
        np.ascontiguousarray(dout.astype(np.float32)),
    )


def kernel(**inputs) -> np.ndarray:
    global LAST_RESULTS, LAST_IN_MAPS

    f32 = lambda a: np.ascontiguousarray(np.asarray(a, dtype=np.float32))
    bf = lambda a: np.ascontiguousarray(
        np.asarray(a, dtype=np.float32).astype(ml_dtypes.bfloat16)
    )

    x = f32(inputs["x"])                       # [8, 1024, 768]
    mask = np.asarray(inputs["attn_mask"])     # [8, 1024] int32

    lng1, lnb1 = f32(inputs["ln1_g"]), f32(inputs["ln1_b"])
    lng2, lnb2 = f32(inputs["ln2_g"]), f32(inputs["ln2_b"])
    ba, bp = f32(inputs["b_attn"]), f32(inputs["b_proj"])
    bf_, bf2 = f32(inputs["b_fc"]), f32(inputs["b_fc2"])
    trivial = bool(
        (lng1 == 1).all() and (lnb1 == 0).all() and (lng2 == 1).all()
        and (lnb2 == 0).all() and (ba == 0).all() and (bp == 0).all()
        and (bf_ == 0).all() and (bf2 == 0).all()
    )

    # Key compaction: permute tokens per batch so unmasked keys come first.
    mask01 = (mask != 0)
    counts = mask01.sum(axis=1)
    compact = bool(counts.max() <= COMPACT_KT * 128)
    kt_chunks = COMPACT_KT if compact else TT

    perms = []
    for b in range(N_CORES):
        perm = np.argsort(~mask01[b], kind="stable")  # unmasked first
        perms.append(perm)

    nc = _get_nc(trivial, kt_chunks)

    if trivial:
        cvec, dvec, w12_bf, dout = _mlp_linearization(
            f32(inputs["W_fc"]), f32(inputs["W_fc2"])
        )
        common = {
            "w_attn": _arr(_f8(f32(inputs["W_attn"]), 16.0)),
            "w_proj": _arr(_f8(f32(inputs["W_proj"]), 16.0)),
            "w_fc": _arr(_f8(f32(inputs["W_fc"]), 16.0)),
            "w_fc2": _arr(_f8(f32(inputs["W_fc2"]), 32.0)),
            "w12": _arr(w12_bf),
            "cvec": _col(cvec),
            "dvec": _col(dvec),
            "dout": dout,
        }
    else:
        common = {
            "w_attn": bf(inputs["W_attn"]),
            "w_proj": bf(inputs["W_proj"]),
            "w_fc": bf(inputs["W_fc"]),
            "w_fc2": bf(inputs["W_fc2"]),
            "ln1_g": lng1, "ln1_b": lnb1, "ln2_g": lng2, "ln2_b": lnb2,
            "b_attn": ba, "b_proj": bp, "b_fc": bf_, "b_fc2": bf2,
        }
    in_maps = []
    for b in range(N_CORES):
        m = dict(common)
        m["x"] = np.ascontiguousarray(x[b][perms[b]])
        m01 = mask01[b][perms[b]].astype(np.float32)
        if trivial:
            m01 = np.ascontiguousarray(m01.reshape(TT, 128).T)
        m["mask01"] = np.ascontiguousarray(m01)
        in_maps.append(m)

    from concourse.bass_utils import run_bass_kernel_spmd

    LAST_IN_MAPS = in_maps
    res = run_bass_kernel_spmd(nc, in_maps, core_ids=list(range(N_CORES)), trace=TRACE)
    LAST_RESULTS = res
    out = np.empty((N_CORES, T, C), np.float32)
    for b in range(N_CORES):
        out[b, perms[b]] = np.asarray(res.results[b]["out"])
    return out

